# revision 42
# baseline (speedup 1.0000x reference)
"""Trainium2 Bass kernel v3 for a pre-norm transformer block.

Data-parallel B=8 over 8 cores. Per-core, activations transposed [feat, tok].

v3 vs v2: token-half software pipeline so the ACT-bound attention exp
overlaps the PE-bound MLP matmuls, keeping the PE dense (HAM warm):
  ph1: LN1+qk per half (rstd chain of half b hides under qk of half a)
  ph2: v matmuls || scores+exp(half a) prefetch, per-head attn(a)
  ph3: per-head attn(b) || proj(a), LN2(a), fc1(a) matmuls (pre-gelu
       staged to bf16 SBUF so no exp<->gelu ACT-table thrash)
  ph4: proj(b), LN2(b), gelu(a) burst || fc2(a) progressive,
       fc1(b)+fused gelu, fc2(b), residual + out
rstd via single ACT Rsqrt (no Ln/Exp table ping-pong). LN stats squares
on ACT Square (resident in every table set). Softmax denominator
reciprocal taken straight from the PSUM row, K=1 PE broadcast.
"""

import numpy as np
import ml_dtypes

EMBED = 1024
HEADS = 16
HIDDEN = 4096
N_TOK = 1024
B = 8
N_CORES = 8
EPS = 1e-5
P = 128
CSUB = EMBED // P          # 8
HSUB = HIDDEN // P         # 32
QW = 512                   # token half width
NHALF = 2

F8 = ml_dtypes.float8_e4m3
WS = 256.0                 # weight scale for qkv(qk)/proj/fc1/fc2
VS = 64.0                  # weight scale for v path (ones col = 64 cancels)

_CACHE = {}
GELU = True
TAPS = False     # debug: extra dram taps


# ---------------------------------------------------------------------------
# host-side packing (unchanged from v2)
# ---------------------------------------------------------------------------

def _pack_dr(w, scale):
    """[K, M] fp32 -> [M//128, 128, K//256, 2, 128] fp8 DoubleRow chunks."""
    K, M = w.shape
    a = w.reshape(K // 256, 2, P, M // P, P).transpose(3, 2, 0, 1, 4)
    return np.ascontiguousarray((a * scale).astype(F8))


def _pack_rhs8(w, scale):
    """[K, M] fp32 -> [128, K//128, M] fp8 (moving layout)."""
    K, M = w.shape
    a = w.reshape(K // P, P, M).transpose(1, 0, 2)
    return np.ascontiguousarray((a * scale).astype(F8))


def _pack_percol(v):
    F = v.shape[0]
    return np.ascontiguousarray(v.reshape(F // P, P).T.astype(np.float32))


def _pack_xT(xb):
    xT = xb.T
    a = xT.reshape(CSUB, P, N_TOK).transpose(1, 0, 2)
    return np.ascontiguousarray(a.astype(np.float32))


def _unpack_yT(yT):
    full = yT.transpose(1, 0, 2).reshape(EMBED, N_TOK)
    return np.ascontiguousarray(full.T)


# ---------------------------------------------------------------------------
# kernel build
# ---------------------------------------------------------------------------

def _build():
    import concourse.bacc as bacc
    import concourse.mybir as mybir
    import concourse.tile as tile
    from contextlib import ExitStack

    dt = mybir.dt
    AF = mybir.ActivationFunctionType
    OP = mybir.AluOpType
    DR = mybir.MatmulPerfMode.DoubleRow

    nc = bacc.Bacc("TRN2", target_bir_lowering=False, debug=False)

    f32, bf16, f8 = dt.float32, dt.bfloat16, dt.float8e4

    def dram(name, shape, d=f32, out=False):
        return nc.dram_tensor(name, list(shape), d,
                              kind="ExternalOutput" if out else "ExternalInput").ap()

    xT_d = dram("xT", [P, CSUB, N_TOK])
    wqk_d = dram("wqk", [16, P, 4, 2, P], f8)
    bqk_d = dram("bqk", [P, 16])
    wv_d = dram("wv", [P, CSUB, EMBED], f8)
    bv_d = dram("bv", [1, EMBED], bf16)
    wpr_d = dram("wpr", [CSUB, P, 4, 2, P], f8)
    bpr_d = dram("bpr", [P, CSUB])
    wf1_d = dram("wf1", [HSUB, P, 4, 2, P], f8)
    bf1_d = dram("bf1", [P, HSUB])
    wf2_d = dram("wf2", [CSUB, P, 16, 2, P], f8)
    bf2_d = dram("bf2", [P, CSUB])
    g1_d = dram("g1", [P, CSUB])
    g2_d = dram("g2", [P, CSUB])
    yT_d = dram("yT", [P, CSUB, N_TOK], out=True)
    if TAPS:
        mu_t_d = dram("mu_t", [P, N_TOK], out=True)
        rstd_t_d = dram("rstd_t", [P, N_TOK], out=True)
        hT_t_d = dram("hT_t", [P, CSUB, N_TOK], f8, out=True)
        e0_t_d = dram("e0_t", [P, 2, QW], f8, out=True)
        oU_t_d = dram("oU_t", [64, QW], bf16, out=True)
        r65_t_d = dram("r65_t", [1, QW], out=True)
        qkp_t_d = dram("qkp_t", [P, 8, 2, N_TOK], bf16, out=True)
        v65_t_d = dram("v65_t", [P, CSUB, HEADS, 65], f8, out=True)
        oT_t_d = dram("oT_t", [P, CSUB, N_TOK], f8, out=True)
        ln2_t_d = dram("ln2_t", [P, CSUB, N_TOK], f8, out=True)
        f1s_t_d = dram("f1s_t", [P, HSUB, QW], bf16, out=True)
        gel_t_d = dram("gel_t", [P, HSUB, N_TOK], f8, out=True)

    HSL = [slice(0, QW), slice(QW, N_TOK)]

    with tile.TileContext(nc) as tc, ExitStack() as ctx:
        const = ctx.enter_context(tc.tile_pool(name="const", bufs=1))
        persist = ctx.enter_context(tc.tile_pool(name="persist", bufs=1))
        smalls = ctx.enter_context(tc.tile_pool(name="smalls", bufs=1))
        tmpf = ctx.enter_context(tc.tile_pool(name="tmpf", bufs=2))
        wpool = ctx.enter_context(tc.tile_pool(name="wpool", bufs=2))
        # spans phases 3-4
        mlpP = ctx.enter_context(tc.tile_pool(name="mlpP", bufs=1))
        # oT spans phases 2-4
        attn_sb = ctx.enter_context(tc.tile_pool(name="attn_sb", bufs=1))

        # ---- constants ---------------------------------------------------
        ones_mm = const.tile([P, P], bf16)      # 1/1024 for LN mean
        nc.vector.memset(ones_mm[:], 1.0 / EMBED)
        ones_bc = const.tile([65, P], bf16)     # K=1 broadcast rows
        nc.vector.memset(ones_bc[:], 1.0)


        bqk_sb = const.tile([P, 16], f32)
        nc.sync.dma_start(bqk_sb[:], bqk_d[:])
        bv_row = const.tile([1, EMBED], bf16)
        nc.sync.dma_start(bv_row[:], bv_d[:])
        bpr_sb = const.tile([P, CSUB], f32)
        nc.sync.dma_start(bpr_sb[:], bpr_d[:])
        bf1_sb = const.tile([P, HSUB], f32)
        nc.sync.dma_start(bf1_sb[:], bf1_d[:])
        bf2_sb = const.tile([P, CSUB], f32)
        nc.sync.dma_start(bf2_sb[:], bf2_d[:])
        g1_sb = const.tile([P, CSUB], f32)
        nc.sync.dma_start(g1_sb[:], g1_d[:])
        g2_sb = const.tile([P, CSUB], f32)
        nc.sync.dma_start(g2_sb[:], g2_d[:])

        xT = persist.tile([P, CSUB, N_TOK], f32)
        for c in range(CSUB):
            for half in range(NHALF):
                nc.sync.dma_start(xT[:, c, HSL[half]], xT_d[:, c, HSL[half]])

        oT = attn_sb.tile([P, CSUB, N_TOK], f8)
        ln2T = mlpP.tile([P, CSUB, N_TOK], f8)
        f1stage = mlpP.tile([P, HSUB, QW], bf16)

        mu_sb = smalls.tile([P, N_TOK], f32)
        rstd = smalls.tile([P, N_TOK], f32)

        # ---- LN building blocks -----------------------------------------
        def stage_xsq(xsq_pair, x_sb, c, half):
            """Stage x and x^2 interleaved (bf16) so mu/sq stats are ONE
            matmul accumulation chain per quarter (single PSUM bank;
            avoids the whole-bank has_written clear of a second chain)."""
            for qq in range(2):
                q0 = half * QW + qq * 256
                src = x_sb[:, c, q0:q0 + 256]
                nc.vector.tensor_copy(xsq_pair[qq][:, c, 0, :], src)
                nc.scalar.activation(xsq_pair[qq][:, c, 1, :], src,
                                     AF.Square)

        def stats_from_xsq(stp, xsq_pair, half, rstd_t, mu_t):
            """Quarter-sequenced LN stats in one PSUM bank, chained to
            rstd (ACT Sqrt + DVE reciprocal) and mu."""
            for qq in range(2):
                gq = half * QW + qq * 256
                st = stp.tile([P, 2, 256], f32, tag="st")
                for c in range(CSUB):
                    nc.tensor.matmul(st[:, :, :], ones_mm[:],
                                     xsq_pair[qq][:, c, :, :],
                                     start=(c == 0), stop=(c == CSUB - 1))
                nc.vector.tensor_copy(mu_t[:, gq:gq + 256], st[:, 0, :])
                mu2 = tmpf.tile([P, 256], f32, tag="mu2", bufs=2)
                nc.scalar.activation(mu2[:], st[:, 0, :], AF.Square)
                var_t = tmpf.tile([P, 256], f32, tag="var", bufs=2)
                nc.vector.scalar_tensor_tensor(var_t[:], st[:, 1, :],
                                               EPS, mu2[:],
                                               OP.add, OP.subtract)
                std_t = tmpf.tile([P, 256], f32, tag="std", bufs=2)
                nc.scalar.activation(std_t[:], var_t[:], AF.Sqrt)
                nc.vector.reciprocal_approx_fast(rstd_t[:, gq:gq + 256],
                                                 std_t[:])

        def emit_norm_half(x_sb, g_col, out_sb, half, rstd_t, mu_t,
                           step_cb=None):
            hsl = HSL[half]
            for cp in range(4):
                for c in (2 * cp, 2 * cp + 1):
                    t = tmpf.tile([P, QW], f32, tag="lnt", bufs=2)
                    nc.vector.tensor_tensor(t[:], x_sb[:, c, hsl],
                                            mu_t[:, hsl], OP.subtract)
                    nc.vector.scalar_tensor_tensor(
                        out_sb[:, c, hsl], t[:], g_col[:, c:c + 1],
                        rstd_t[:, hsl], OP.mult, OP.mult)
                if step_cb is not None:
                    step_cb(cp)

        # =================================================================
        # phases 1-3 under hT/qkp lifetime
        # =================================================================
        with tc.tile_pool(name="hTp", bufs=1) as hTp, \
             tc.tile_pool(name="qk_sb", bufs=1) as qk_sb:

            hT = hTp.tile([P, CSUB, N_TOK], f8)
            qkp = qk_sb.tile([P, 8, 2, N_TOK], bf16)  # [d, hp, q/k, tok]

            # ---- phase 1: LN1 + q,k ------------------------------------
            with tc.tile_pool(name="wqkp", bufs=1) as wqkp, \
                 tc.tile_pool(name="lnb", bufs=1) as lnb, \
                 tc.tile_pool(name="warm", bufs=1) as warmp, \
                 tc.tile_pool(name="st1", bufs=1, space="PSUM") as st1, \
                 tc.tile_pool(name="wps", bufs=1, space="PSUM") as wps, \
                 tc.tile_pool(name="psQ", bufs=4, space="PSUM") as psQ:

                # HAM warmup: keep the PE busy with dummy matmuls while
                # the xT DMA + LN staging run, so the first real matmuls
                # execute at K=8/8 (2.4 GHz) instead of cold 1.2 GHz.
                wsrc = warmp.tile([P, QW], bf16)
                nc.vector.memset(wsrc[:], 0.0)
                w_ps = wps.tile([P, QW], f32)
                for _ in range(24):
                    nc.tensor.matmul(w_ps[:], ones_mm[:], wsrc[:],
                                     start=True, stop=True)

                wqk_sb = []
                for m in range(16):
                    w = wqkp.tile([P, 4, 2, P], f8, name=f"wqk{m}")
                    nc.gpsimd.dma_start(w[:], wqk_d[m])
                    wqk_sb.append(w)

                def qk_mtile(m, half):
                    hsl = HSL[half]
                    p_ps = psQ.tile([P, QW], f32, tag="ps")
                    for kp in range(4):
                        nc.tensor.matmul(p_ps[:], wqk_sb[m][:, kp, :, :],
                                         hT[:, 2 * kp:2 * kp + 2, hsl],
                                         start=(kp == 0), stop=(kp == 3),
                                         perf_mode=DR)
                    nc.scalar.activation(qkp[:, m % 8, m // 8, hsl],
                                         p_ps[:], AF.Identity,
                                         bias=bqk_sb[:, m:m + 1])

                PRE_M = (0, 8, 1, 9)

                def make_qk_cb(half):
                    hsl = HSL[half]
                    pre_ps = []

                    def cb(cp):
                        if cp == 0:
                            for i in range(4):
                                pre_ps.append(psQ.tile(
                                    [P, QW], f32, tag="ps",
                                    name=f"pq{half}{i}"))
                        for i, m in enumerate(PRE_M):
                            nc.tensor.matmul(
                                pre_ps[i][:], wqk_sb[m][:, cp, :, :],
                                hT[:, 2 * cp:2 * cp + 2, hsl],
                                start=(cp == 0), stop=(cp == 3),
                                perf_mode=DR)
                        if cp == 3:
                            for i, m in enumerate(PRE_M):
                                nc.scalar.activation(
                                    qkp[:, m % 8, m // 8, hsl],
                                    pre_ps[i][:], AF.Identity,
                                    bias=bqk_sb[:, m:m + 1])
                    return cb

                for half in range(NHALF):
                    xsq_pair = [lnb.tile([P, CSUB, 2, 256], bf16,
                                         tag=f"xsq{qq}", name=f"xsq1_{qq}")
                                for qq in range(2)]
                    for c in range(CSUB):
                        stage_xsq(xsq_pair, xT, c, half)
                    stats_from_xsq(st1, xsq_pair, half, rstd, mu_sb)
                    emit_norm_half(xT, g1_sb, hT, half, rstd, mu_sb,
                                   step_cb=make_qk_cb(half))
                    for m in range(16):
                        if m not in PRE_M:
                            qk_mtile(m, half)

                # residual pre-bias: xT += bpr, after LN1 consumed xT
                for c in range(CSUB):
                    nc.vector.tensor_scalar(xT[:, c, :], xT[:, c, :],
                                            bpr_sb[:, c:c + 1], None,
                                            OP.add)
                if TAPS:
                    nc.sync.dma_start(mu_t_d[:], mu_sb[:])
                    nc.sync.dma_start(rstd_t_d[:], rstd[:])
                    nc.sync.dma_start(hT_t_d[:], hT[:])
                    nc.sync.dma_start(qkp_t_d[:], qkp[:])

            # ---- phases 2-3: attention + overlapped MLP(a) --------------
            with tc.tile_pool(name="v65p", bufs=1) as v65p, \
                 tc.tile_pool(name="sp", bufs=2, space="PSUM") as sp, \
                 tc.tile_pool(name="obc", bufs=1, space="PSUM") as obc, \
                 tc.tile_pool(name="epool", bufs=16) as epool, \
                 tc.tile_pool(name="onorm", bufs=3) as onorm:

                v65 = v65p.tile([P, CSUB, HEADS, 65], f8)
                nc.vector.memset(v65[:, :, :, 64:65], 64.0)

                def scores_exp(h, half):
                    """scores + exp for one head/half; safe to emit before
                    v65 is complete. Returns the 4 e tiles."""
                    hp = h // 2
                    bs = slice((h % 2) * 64, (h % 2) * 64 + 64)
                    hsl = HSL[half]
                    e_list = []
                    for tp in range(4):
                        s_pair = sp.tile([P, 2, QW], f32, tag="sp")
                        for j in range(2):
                            k = 2 * tp + j
                            nc.tensor.matmul(
                                s_pair[:, j, :],
                                qkp[bs, hp, 1, k * P:(k + 1) * P],
                                qkp[bs, hp, 0, hsl])
                        e_t = epool.tile([P, 2, QW], f8, tag="exp")
                        nc.scalar.activation(e_t[:], s_pair[:], AF.Exp,
                                             scale=0.125 / (WS * WS))
                        if TAPS and h == 0 and half == 0 and tp == 0:
                            nc.sync.dma_start(e0_t_d[:], e_t[:])
                        e_list.append(e_t)
                    return e_list

                def attnV_norm(h, half, e_list):
                    """attnV + softmax normalize; requires v65 fully
                    emitted (reads all 8 token blocks)."""
                    hsl = HSL[half]
                    o_ps = obc.tile([65, QW], f32, tag="obc", name="ops")
                    for tp in range(4):
                        nc.tensor.matmul(o_ps[:],
                                         v65[:, 2 * tp:2 * tp + 2, h, :],
                                         e_list[tp][:], start=(tp == 0),
                                         stop=(tp == 3), perf_mode=DR)
                    oU65 = onorm.tile([65, QW], bf16, tag="oU")
                    nc.vector.tensor_copy(oU65[:], o_ps[:])
                    bc_ps = obc.tile([64, QW], f32, tag="obc", name="bcps")
                    nc.tensor.matmul(bc_ps[:], ones_bc[64:65, 0:64],
                                     oU65[64:65, :])
                    bc_sb = onorm.tile([64, QW], f32, tag="bcsb", bufs=2)
                    nc.vector.reciprocal_approx_fast(bc_sb[:], bc_ps[:])
                    if TAPS and h == 0 and half == 0:
                        nc.sync.dma_start(oU_t_d[:], oU65[0:64, :])
                        nc.sync.dma_start(r65_t_d[:], bc_sb[0:1, :])
                    if h % 2 == 0:
                        nc.vector.tensor_tensor(oT[0:64, h // 2, hsl],
                                                oU65[0:64, :], bc_sb[:],
                                                OP.mult)
                    else:
                        n64 = onorm.tile([64, QW], f8, tag="n64")
                        nc.vector.tensor_tensor(n64[:], oU65[0:64, :],
                                                bc_sb[:], OP.mult)
                        nc.sync.dma_start(oT[64:128, h // 2, hsl], n64[:])

                def attn_head(h, half):
                    attnV_norm(h, half, scores_exp(h, half))

                # ---- phase 2: v || attn(a) ------------------------------
                with tc.tile_pool(name="wv_sb", bufs=1) as wvp, \
                     tc.tile_pool(name="vps", bufs=2, space="PSUM") as vps:

                    wv_sb = wvp.tile([P, CSUB, EMBED], f8)
                    nc.gpsimd.dma_start(wv_sb[:], wv_d[:])
                    vb_ps = vps.tile([P, QW], f32, tag="vp", name="vbps")
                    nc.tensor.matmul(vb_ps[:], ones_bc[0:1, :],
                                     bv_row[:, 0:QW])
                    vb_ps2 = vps.tile([P, QW], f32, tag="vp", name="vbps2")
                    nc.tensor.matmul(vb_ps2[:], ones_bc[0:1, :],
                                     bv_row[:, QW:EMBED])
                    vb_b = wvp.tile([P, EMBED], f32)
                    nc.vector.tensor_copy(vb_b[:, 0:QW], vb_ps[:])
                    nc.vector.tensor_copy(vb_b[:, QW:EMBED], vb_ps2[:])

                    def v_mtile(mt, fh):
                        """token block mt, feature half fh (8 heads)."""
                        fsl = slice(fh * QW, (fh + 1) * QW)
                        v_ps = vps.tile([P, QW], f32, tag="vp")
                        for kp in range(4):
                            nc.tensor.matmul(
                                v_ps[:],
                                hT[:, 2 * kp:2 * kp + 2,
                                   mt * P:(mt + 1) * P],
                                wv_sb[:, 2 * kp:2 * kp + 2, fsl],
                                start=(kp == 0), stop=(kp == 3),
                                perf_mode=DR)
                        nc.vector.tensor_tensor(
                            v65[:, mt, fh * 8:(fh + 1) * 8, 0:64],
                            v_ps[:].rearrange("p (h d) -> p h d", d=64),
                            vb_b[:, fsl].rearrange("p (h d) -> p h d",
                                                   d=64),
                            OP.add)

                    # prefetch scores+exp for early half-a heads during v
                    # (attnV must wait: it reads all 8 v65 token blocks)
                    pre_e = {}
                    i = 0
                    for mt in range(CSUB):
                        for fh in range(2):
                            v_mtile(mt, fh)
                            if i % 5 == 4 and len(pre_e) < 3:
                                h = len(pre_e)
                                pre_e[h] = scores_exp(h, 0)
                            i += 1

                for h in range(HEADS):
                    if h in pre_e:
                        attnV_norm(h, 0, pre_e.pop(h))
                    else:
                        attn_head(h, 0)

                # ---- phase 3: attn(b) || proj(a)/LN2(a)/fc1(a) ----------
                with tc.tile_pool(name="wprp", bufs=1) as wprp, \
                     tc.tile_pool(name="ln2b", bufs=1) as ln2b, \
                     tc.tile_pool(name="psM", bufs=2, space="PSUM") as psM, \
                     tc.tile_pool(name="st2", bufs=1, space="PSUM") as st2:

                    wpr_sb = []
                    for m in range(CSUB):
                        w = wprp.tile([P, 4, 2, P], f8, name=f"wpr{m}")
                        nc.gpsimd.dma_start(w[:], wpr_d[m])
                        wpr_sb.append(w)

                    xsq3 = [ln2b.tile([P, CSUB, 2, 256], bf16,
                                      name=f"xsq3{qq}") for qq in range(2)]

                    def proj_mtile(m, half):
                        hsl = HSL[half]
                        p_ps = psM.tile([P, QW], f32, tag="ps")
                        for kp in range(4):
                            nc.tensor.matmul(
                                p_ps[:], wpr_sb[m][:, kp, :, :],
                                oT[:, 2 * kp:2 * kp + 2, hsl],
                                start=(kp == 0), stop=(kp == 3),
                                perf_mode=DR)
                        nc.vector.scalar_tensor_tensor(
                            xT[:, m, hsl], p_ps[:], 1.0 / WS,
                            xT[:, m, hsl], OP.mult, OP.add)
                        stage_xsq(xsq3, xT, m, half)

                    wf1_a = []
                    for m in range(HSUB):
                        w = wpool.tile([P, 4, 2, P], f8, tag="wf1k",
                                       bufs=6, name=f"wfa{m}")
                        nc.gpsimd.dma_start(w[:], wf1_d[m])
                        wf1_a.append(w)

                    def fc1_mtile_stage(m):
                        """fc1 m-tile for half a; stage pre-gelu bf16."""
                        f_ps = psM.tile([P, QW], f32, tag="ps")
                        for kp in range(4):
                            nc.tensor.matmul(
                                f_ps[:], wf1_a[m][:, kp, :, :],
                                ln2T[:, 2 * kp:2 * kp + 2, HSL[0]],
                                start=(kp == 0), stop=(kp == 3),
                                perf_mode=DR)
                        nc.vector.tensor_copy(f1stage[:, m, :], f_ps[:])

                    FC1_PRE = (0, 1)

                    def fc1_pre_cb(cp):
                        """feed fc1(a) pre-tiles during LN2(a) normalize"""
                        if cp == 0:
                            fc1_pre_ps.clear()
                            for m in FC1_PRE:
                                fc1_pre_ps.append(psM.tile(
                                    [P, QW], f32, tag="ps",
                                    name=f"f1p{m}"))
                        for i, m in enumerate(FC1_PRE):
                            nc.tensor.matmul(
                                fc1_pre_ps[i][:], wf1_a[m][:, cp, :, :],
                                ln2T[:, 2 * cp:2 * cp + 2, HSL[0]],
                                start=(cp == 0), stop=(cp == 3),
                                perf_mode=DR)
                        if cp == 3:
                            for i, m in enumerate(FC1_PRE):
                                nc.vector.tensor_copy(f1stage[:, m, :],
                                                      fc1_pre_ps[i][:])

                    fc1_pre_ps = []
                    mlp_work = []
                    for m in range(CSUB):
                        mlp_work.append(lambda m=m: proj_mtile(m, 0))
                    mlp_work.append(lambda: stats_from_xsq(
                        st2, xsq3, 0, rstd, mu_sb))
                    mlp_work.append(lambda: emit_norm_half(
                        xT, g2_sb, ln2T, 0, rstd, mu_sb,
                        step_cb=fc1_pre_cb))
                    for m in range(HSUB):
                        if m not in FC1_PRE:
                            mlp_work.append(lambda m=m: fc1_mtile_stage(m))

                    wi = 0
                    for h in range(HEADS):
                        attn_head(h, 1)
                        target = ((h + 1) * len(mlp_work)) // HEADS
                        while wi < target:
                            mlp_work[wi]()
                            wi += 1
                    if TAPS:
                        nc.sync.dma_start(v65_t_d[:], v65[:])
                        nc.sync.dma_start(oT_t_d[:], oT[:])
                        nc.sync.dma_start(f1s_t_d[:], f1stage[:])

        # =================================================================
        # phase 4: proj(b), LN2(b), gelu(a) || fc2(a), fc1(b)+gelu, fc2(b)
        # =================================================================
        with tc.tile_pool(name="wprp2", bufs=1) as wprp2, \
             tc.tile_pool(name="gelup", bufs=1) as gelup, \
             tc.tile_pool(name="ln2c", bufs=1) as ln2c, \
             tc.tile_pool(name="w4k", bufs=4) as w4k, \
             tc.tile_pool(name="ps4a", bufs=4, space="PSUM") as ps4a, \
             tc.tile_pool(name="psN", bufs=2, space="PSUM") as psN, \
             tc.tile_pool(name="st3", bufs=1, space="PSUM") as st3:

            geluT = gelup.tile([P, HSUB, N_TOK], f8)

            wpr2_sb = []
            for m in range(CSUB):
                w = wprp2.tile([P, 4, 2, P], f8, name=f"wpr2{m}")
                nc.gpsimd.dma_start(w[:], wpr_d[m])
                wpr2_sb.append(w)

            xsq4 = [ln2c.tile([P, CSUB, 2, 256], bf16,
                              name=f"xsq4{qq}") for qq in range(2)]

            # proj(b) + LN2(b) staging
            for m in range(CSUB):
                p_ps = psN.tile([P, QW], f32, tag="ps")
                for kp in range(4):
                    nc.tensor.matmul(p_ps[:], wpr2_sb[m][:, kp, :, :],
                                     oT[:, 2 * kp:2 * kp + 2, HSL[1]],
                                     start=(kp == 0), stop=(kp == 3),
                                     perf_mode=DR)
                nc.vector.scalar_tensor_tensor(
                    xT[:, m, HSL[1]], p_ps[:], 1.0 / WS,
                    xT[:, m, HSL[1]], OP.mult, OP.add)
                stage_xsq(xsq4, xT, m, 1)
            stats_from_xsq(st3, xsq4, 1, rstd, mu_sb)

            # residual pre-bias for fc2, half a only (half b after its
            # LN2 normalize has consumed xT)
            for c in range(CSUB):
                nc.vector.tensor_scalar(xT[:, c, HSL[0]], xT[:, c, HSL[0]],
                                        bf2_sb[:, c:c + 1], None, OP.add)

            # gelu(a) burst from staged bf16 (single switch to gelu set)
            for m in range(HSUB):
                nc.scalar.activation(geluT[:, m, HSL[0]], f1stage[:, m, :],
                                     AF.Gelu if GELU else AF.Identity,
                                     bias=bf1_sb[:, m:m + 1],
                                     scale=1.0 / WS)

            # fc2(a): two groups of 4 m-tiles, kp-progressive so the PE
            # follows the gelu(a) burst as chunks land
            w2_sb = []
            for m2 in range(CSUB):
                w = w4k.tile([P, 16, 2, P], f8, tag="w4k", bufs=4,
                             name=f"w2{m2 % 4}")
                nc.gpsimd.dma_start(w[:], wf2_d[m2])
                w2_sb.append(w)

            def fc2_group(ms, half, wlist):
                hsl = HSL[half]
                y_list = [ps4a.tile([P, QW], f32, tag="ps",
                                    name=f"y{half}{m2 % 4}") for m2 in ms]
                for kp in range(16):
                    for y_ps, m2 in zip(y_list, ms):
                        nc.tensor.matmul(
                            y_ps[:], wlist[m2][:, kp, :, :],
                            geluT[:, 2 * kp:2 * kp + 2, hsl],
                            start=(kp == 0), stop=(kp == 15),
                            perf_mode=DR)
                for y_ps, m2 in zip(y_list, ms):
                    nc.vector.scalar_tensor_tensor(
                        xT[:, m2, hsl], y_ps[:], 1.0 / WS,
                        xT[:, m2, hsl], OP.mult, OP.add)
                    nc.sync.dma_start(yT_d[:, m2, hsl], xT[:, m2, hsl])

            fc2_group([0, 1, 2, 3], 0, w2_sb)

            # LN2(b) normalize + fc1(b) with fused gelu eviction
            wf1_b = []
            for m in range(HSUB):
                w = wpool.tile([P, 4, 2, P], f8, tag="wf1k", bufs=6,
                               name=f"wfb{m}")
                nc.gpsimd.dma_start(w[:], wf1_d[m])
                wf1_b.append(w)

            fc1b_pre_ps = []

            def fc1b_pre_cb(cp):
                if cp == 0:
                    for m in FC1_PRE:
                        fc1b_pre_ps.append(psN.tile([P, QW], f32,
                                                    tag="ps",
                                                    name=f"f1bp{m}"))
                for i, m in enumerate(FC1_PRE):
                    nc.tensor.matmul(
                        fc1b_pre_ps[i][:], wf1_b[m][:, cp, :, :],
                        ln2T[:, 2 * cp:2 * cp + 2, HSL[1]],
                        start=(cp == 0), stop=(cp == 3), perf_mode=DR)
                if cp == 3:
                    for i, m in enumerate(FC1_PRE):
                        nc.scalar.activation(
                            geluT[:, m, HSL[1]], fc1b_pre_ps[i][:],
                            AF.Gelu if GELU else AF.Identity,
                            bias=bf1_sb[:, m:m + 1], scale=1.0 / WS)

            emit_norm_half(xT, g2_sb, ln2T, 1, rstd, mu_sb,
                           step_cb=fc1b_pre_cb)
            # residual pre-bias for fc2, half b (after normalize read xT)
            for c in range(CSUB):
                nc.vector.tensor_scalar(xT[:, c, HSL[1]], xT[:, c, HSL[1]],
                                        bf2_sb[:, c:c + 1], None, OP.add)
            for m in range(HSUB):
                if m in FC1_PRE:
                    continue
                f_ps = psN.tile([P, QW], f32, tag="ps")
                for kp in range(4):
                    nc.tensor.matmul(f_ps[:], wf1_b[m][:, kp, :, :],
                                     ln2T[:, 2 * kp:2 * kp + 2, HSL[1]],
                                     start=(kp == 0), stop=(kp == 3),
                                     perf_mode=DR)
                nc.scalar.activation(geluT[:, m, HSL[1]], f_ps[:],
                                     AF.Gelu if GELU else AF.Identity,
                                     bias=bf1_sb[:, m:m + 1],
                                     scale=1.0 / WS)

            fc2_group([4, 5, 6, 7], 0, w2_sb)

            # fc2(b): reload weights (w4k pool rotation) and run both
            # groups back-to-back
            w2b_sb = []
            for m2 in range(CSUB):
                w = w4k.tile([P, 16, 2, P], f8, tag="w4k", bufs=4,
                             name=f"w2b{m2 % 4}")
                nc.gpsimd.dma_start(w[:], wf2_d[m2])
                w2b_sb.append(w)
            fc2_group([0, 1, 2, 3], 1, w2b_sb)
            fc2_group([4, 5, 6, 7], 1, w2b_sb)
            if TAPS:
                nc.sync.dma_start(ln2_t_d[:], ln2T[:])
                nc.sync.dma_start(gel_t_d[:], geluT[:])

    nc.compile()
    return nc


def get_nc():
    if "nc" not in _CACHE:
        _CACHE["nc"] = _build()
    return _CACHE["nc"]


def make_in_maps(x, qkv_w, qkv_b, proj_w, proj_b, fc1_w, fc1_b, fc2_w, fc2_b,
                 ln1_g, ln1_b, ln2_g, ln2_b):
    x = np.asarray(x, np.float32)
    qkv_w = np.asarray(qkv_w, np.float32)
    qkv_b = np.asarray(qkv_b, np.float32)
    fc1_w = np.asarray(fc1_w, np.float32)
    ln1_b = np.asarray(ln1_b, np.float32)
    ln2_b = np.asarray(ln2_b, np.float32)
    # fold LN betas into downstream effective biases (h = (x-mu)*rstd*g dev)
    bqk_eff = (qkv_b[:2048] + ln1_b @ qkv_w[:, :2048]) * WS
    bv_eff = (qkv_b[2048:] + ln1_b @ qkv_w[:, 2048:]) * VS
    bf1_eff = np.asarray(fc1_b, np.float32) + ln2_b @ fc1_w
    shared = {
        "wqk": _pack_dr(qkv_w[:, :2048], WS),
        "bqk": _pack_percol(bqk_eff),
        "wv": _pack_rhs8(qkv_w[:, 2048:], VS),
        "bv": np.ascontiguousarray(bv_eff[None, :].astype(ml_dtypes.bfloat16)),
        "wpr": _pack_dr(np.asarray(proj_w, np.float32), WS),
        "bpr": _pack_percol(np.asarray(proj_b, np.float32)),
        "wf1": _pack_dr(fc1_w, WS),
        "bf1": _pack_percol(bf1_eff),
        "wf2": _pack_dr(np.asarray(fc2_w, np.float32), WS),
        "bf2": _pack_percol(np.asarray(fc2_b, np.float32)),
        "g1": _pack_percol(np.asarray(ln1_g, np.float32)),
        "g2": _pack_percol(np.asarray(ln2_g, np.float32)),
    }
    return [dict(shared, xT=_pack_xT(x[b])) for b in range(B)]


def kernel(**inputs):
    from concourse.bass_utils import run_bass_kernel_spmd

    nc = get_nc()
    in_maps = make_in_maps(**inputs)
    res = run_bass_kernel_spmd(nc, in_maps, core_ids=list(range(N_CORES)))
    out = np.stack([_unpack_yT(res.results[b]["yT"]) for b in range(B)])
    return out.astype(np.float32)
